# revision 46
# baseline (speedup 1.0000x reference)
"""Causal self-attention Trainium2 Bass kernel (bf16, causal-exact, model-driven schedule).

Problem: B=2, N=2048, H=16 heads, Dh=64, D=1024, fp32 in/out.
  qkv = x @ W_qkv; causal softmax(q k^T / sqrt(Dh)) @ v.

Sharding (8 cores): data-parallel on B (2) x tensor-parallel on head groups (4).
Core c handles batch b = c // 4 and heads hg*4 .. hg*4+3 where hg = c % 4.

Inputs are converted to bf16 AND pre-blocked on the host so every DMA is a
contiguous run per SBUF partition (128 descriptors of 4-8KB instead of 1024 of
1KB); weights ride the ACT hardware DGE queue while x chunks ride Sync's, so
the two DMA queues transfer in parallel during the prologue. PE matmul inputs
are bf16 (same 1 col/cycle streaming rate as f32r but fast-weight-load works
and SBUF/DMA halve; fp8 was measured numerically unacceptable: S-fp8 3.1e-2,
AV-fp8 1.9e-1 vs the 2e-2 gate). PSUM accumulation stays fp32.
Per-core layouts (no device transposes):
  xt  [128, 4, 8, 512] bf16; [p, c, t, :] = x[b].T[t*128+p, c*512:(c+1)*512]
  wq/wk/wv [128, 8, 256] bf16; [p, t, :] = W_slice[t*128+p, :]
  outT [256, 2048] f32; row h*64+d, col i = out[b, i, hg*256 + h*64 + d]

Device algorithm per core (all matmul inputs bf16, PSUM fp32):
  qT/kT [dh, i] via matmul(lhsT=W-slice, rhs=xT), PSUM -> bf16 SBUF
  v     [i, dh] via matmul(lhsT=xT-slice, rhs=Wv), stored v-hat = [v | ones64]
  S^T   [j, i] row-tiled matmul pairs (K=64/head, tile_position (l*64,0)) into a
        [128,2,512] PSUM tile; on diagonal j-tiles only the valid i-range
        (i >= j-tile start) is computed/exp'd/accumulated (causal-exact).
  expS^T via one ACT Exp per j-tile (strided [2,valid] AP), fused 1/sqrt(Dh),
        bf16 out; gpsimd affine_select zeroes the 128x128 diagonal triangle only.
  AV    out^T += vhat^T e^T in PSUM; rows 64:128 accumulate the softmax
        denominator (ones trick); normalization = copy + fast reciprocal + mul.
        No max-subtraction: S ~ N(0,1).

Scheduling: the PE executes strictly in issue order, so build_nc runs a greedy
emission loop over unit streams (S/exp units, AV batches, QKV/v/DMA fillers,
dummy warm matmuls) driven by an inline cost model (PE 2.4GHz warm, ACT 1.2GHz,
ps_s double-buffer backpressure, e-tile pool occupancy, PSUM bank group hand-off,
per-queue DMA completion). Dummy matmuls fill any predicted PE idle so the HAM
clock gate never re-throttles (PE micro-idle windows accumulate over a ~3.4us
monitor window and halve the PE clock; measured as the dominant cost of the
original schedule). The exp stream (ScalarE, 1 elem/cycle/lane, ~77us total)
is the pacing constraint late in the kernel: exp calls per j-tile stay at
[128, 2, valid] granularity - merging pairs of calls to save the 352-cycle
ACT overhead was tried and regressed 40% by killing the S/exp double-buffer.

Measured on HW: 197.1us (prev session baseline) -> 129.3us, rel err 4.9e-3.
PE busy ~112us of which ~99us is the streaming floor for this layout
(QKV 41us + S 29us + AV 29us at 1 result-col/cycle/128-rows, causal-exact).
"""

import numpy as np
import ml_dtypes

import concourse.mybir as mybir
import concourse.tile as tile
from concourse import bacc
from concourse.bass_utils import run_bass_kernel_spmd

F32 = mybir.dt.float32
BF16 = mybir.dt.bfloat16

B = 2
N = 2048
D = 1024
HPC = 4             # heads per core
DH = 64
NCH = 4             # i-chunks of 512
CH = 512
DT = 8              # d-tiles of 128
SCALE = 1.0 / 8.0   # 1/sqrt(64)

# ---- inline schedule model constants (ns) ----
PE_CYC = 1.0 / 2.4
ACT_CYC = 1.0 / 1.2
MM_OH = 14.0        # per-matmul issue overhead
ACT_OH = 352 * ACT_CYC
NBUF_S = 2          # ps_s pool depth (units)
EBUFS = 26          # sb_e pool depth (units)

_CACHED_NC = None


def build_nc(debug_plan=None):
    nc = bacc.Bacc("TRN2", target_bir_lowering=False, debug=False)
    # host pre-blocks inputs so every DMA is contiguous per partition:
    #   xt[p, c, t, :] = x[b].T[t*128+p, c*512:(c+1)*512]
    #   w*[p, t, :]    = W_slice[t*128+p, :]
    xt = nc.dram_tensor("xt", [128, NCH, DT, CH], BF16, kind="ExternalInput").ap()
    wq = nc.dram_tensor("wq", [128, DT, 256], BF16, kind="ExternalInput").ap()
    wk = nc.dram_tensor("wk", [128, DT, 256], BF16, kind="ExternalInput").ap()
    wv = nc.dram_tensor("wv", [128, DT, 256], BF16, kind="ExternalInput").ap()
    outT = nc.dram_tensor("outT", [HPC * DH, N], F32, kind="ExternalOutput").ap()

    with tile.TileContext(nc) as tc:
        with (
            tc.tile_pool(name="sb_w", bufs=1) as sb_w,
            tc.tile_pool(name="sb_x", bufs=3) as sb_x,
            tc.tile_pool(name="sb_qk", bufs=1) as sb_qk,
            tc.tile_pool(name="sb_v", bufs=1) as sb_v,
            tc.tile_pool(name="sb_e", bufs=EBUFS) as sb_e,
            tc.tile_pool(name="sb_n", bufs=6) as sb_n,
            tc.tile_pool(name="ps_av", bufs=2, space="PSUM") as ps_av,
            tc.tile_pool(name="ps_qkv", bufs=2, space="PSUM") as ps_qkv,
            tc.tile_pool(name="ps_s", bufs=NBUF_S, space="PSUM") as ps_s,
        ):
            # ---------- persistent SBUF ----------
            wq_sb = sb_w.tile([128, DT, 256], BF16)
            wk_sb = sb_w.tile([128, DT, 256], BF16)
            wv_sb = sb_w.tile([128, DT, 256], BF16)
            qt_sb = sb_qk.tile([128, 2 * N], BF16)   # pair-major, head l at rows l*64
            kt_sb = sb_qk.tile([128, 2 * N], BF16)
            # v-hat: [slot=(it*4+h)][v|ones][64]
            vh_sb = sb_v.tile([128, 16 * HPC, 2, 64], BF16)

            # zeros for dummy matmuls (memset f32 then cast - memset on bf16 is untested)
            wz = sb_v.tile([128, 1], F32)
            xz = sb_v.tile([128, CH], F32)
            wzr = sb_v.tile([128, 1], BF16)
            xzr = sb_v.tile([128, CH], BF16)

            # ---------- DMA thunks (contiguous-per-partition transfers) ----------
            # weights go out on the ACT hardware DGE queue, x chunks on Sync's:
            # the two queues transfer in parallel during the prologue.
            DMA_US_PER_MB = 5.6   # ~180 GB/s effective per queue
            dma_done = {}

            def _dma_model(queue, key, mbytes):
                t0 = max(M[queue], M['pe'], 7000.0)
                M[queue] = t0 + 600.0 + mbytes * DMA_US_PER_MB * 1000.0
                dma_done[key] = M[queue]

            def dma_w(which):
                w_sb, w_dram = {"q": (wq_sb, wq), "k": (wk_sb, wk), "v": (wv_sb, wv)}[which]
                nc.scalar.dma_start(w_sb[:, :, :], w_dram[:, :, :])
                _dma_model('actq', 'w' + which, 0.5)

            xtc_tiles = {}

            def dma_x(c, halves=False):
                xtc = sb_x.tile([128, DT, CH], BF16, tag="xtc", name=f"xtc{c}")
                xtc_tiles[c] = xtc
                if halves:
                    nc.sync.dma_start(xtc[:, 0:DT // 2, :], xt[:, c, 0:DT // 2, :])
                    _dma_model('synq', f'x{c}a', 0.5)
                    nc.sync.dma_start(xtc[:, DT // 2:DT, :], xt[:, c, DT // 2:DT, :])
                    _dma_model('synq', f'x{c}b', 0.5)
                else:
                    nc.sync.dma_start(xtc[:, :, :], xt[:, c, :, :])
                    _dma_model('synq', f'x{c}a', 1.0)
                    dma_done[f'x{c}b'] = dma_done[f'x{c}a']

            # ---------- schedule model state ----------
            M = dict(pe=0.0, act=0.0, dve=0.0, synq=0.0, actq=0.0, ncold=0)
            sring = []          # exp_done times of S units, for ps_s backpressure
            exp_done = {}       # (p,c,jt) -> modeled exp completion
            e_consumed = {}     # (p,c,jt) -> remaining AV consumers
            avfree = 0.0        # modeled time previous AV bank group is released

            def m_pe(dur):
                M['pe'] += dur

            def m_exp(key, valid):
                st = max(M['pe'], M['act'])
                M['act'] = st + 2 * valid * ACT_CYC + ACT_OH
                exp_done[key] = M['act']
                e_consumed[key] = 2
                sring.append(M['act'])

            # ---------- compute thunks ----------
            def fill_to(t):
                """Emit dummy matmuls to cover a predicted PE stall until time t."""
                stall = t - M['pe']
                if stall > 400.0:
                    dummy(min(24, max(1, int(stall / 380.0))))
                if t > M['pe']:
                    M['pe'] = t

            def qk_piece(c, which, p):
                xtc = xtc_tiles[c]
                w_sb, dst = (wq_sb, qt_sb) if which == "q" else (wk_sb, kt_sb)
                wkey = 'wq' if which == "q" else 'wk'
                a1 = max(dma_done.get(wkey, 0.0), dma_done.get(f'x{c}a', 0.0))
                a2 = dma_done.get(f'x{c}b', 0.0)
                fill_to(a1)
                pres = ps_qkv.tile([128, CH], F32, tag="ps_qkv", name=f"{which}{c}p{p}")
                for t in range(DT // 2):
                    nc.tensor.matmul(
                        pres[:], w_sb[:, t, p * 128:(p + 1) * 128],
                        xtc[:, t, :],
                        start=(t == 0), stop=False)
                m_pe((DT // 2) * (CH * PE_CYC + MM_OH))
                fill_to(a2)
                for t in range(DT // 2, DT):
                    nc.tensor.matmul(
                        pres[:], w_sb[:, t, p * 128:(p + 1) * 128],
                        xtc[:, t, :],
                        start=False, stop=(t == DT - 1))
                m_pe((DT // 2) * (CH * PE_CYC + MM_OH))
                nc.vector.tensor_copy(dst[:, p * N + c * CH: p * N + (c + 1) * CH], pres[:])

            def v_piece(c, il):
                xtc = xtc_tiles[c]
                it = 4 * c + il
                fill_to(max(dma_done.get('wv', 0.0), dma_done.get(f'x{c}b', 0.0)))
                v_ps = ps_qkv.tile([128, 256], F32, tag="ps_qkv", name=f"v{c}i{il}")
                for t in range(DT):
                    nc.tensor.matmul(
                        v_ps[:], xtc[:, t, il * 128:(il + 1) * 128],
                        wv_sb[:, t, :],
                        start=(t == 0), stop=(t == DT - 1))
                for h in range(HPC):
                    nc.vector.tensor_copy(vh_sb[:, it * HPC + h, 0, :],
                                          v_ps[:, h * 64:(h + 1) * 64])
                m_pe(DT * (256 * PE_CYC + MM_OH))

            e_tiles = {}

            def s_exp(p, c, jt):
                jl = jt - 4 * c
                off = 128 * jl if jl > 0 else 0
                valid = CH - off
                s_ps = ps_s.tile([128, 2, CH], F32, tag="ps_s", name=f"s{c}p{p}j{jt}")
                for l in range(2):
                    nc.tensor.matmul(
                        s_ps[:, l, off:],
                        kt_sb[l * 64:(l + 1) * 64, p * N + jt * 128: p * N + (jt + 1) * 128],
                        qt_sb[l * 64:(l + 1) * 64, p * N + c * CH + off: p * N + (c + 1) * CH],
                        start=True, stop=True,
                        tile_position=(l * 64, 0))
                e_t = sb_e.tile([128, 2, CH], BF16, tag="e", name=f"e{c}p{p}j{jt}")
                e_tiles[(p, c, jt)] = e_t
                nc.scalar.activation(e_t[:, :, off:], s_ps[:, :, off:],
                                     mybir.ActivationFunctionType.Exp,
                                     scale=SCALE)
                if jl >= 0:  # diagonal tile: zero where j > i inside the 128-wide square
                    for l in range(2):
                        nc.gpsimd.affine_select(
                            out=e_t[:, l, off:off + 128],
                            in_=e_t[:, l, off:off + 128],
                            compare_op=mybir.AluOpType.is_ge,
                            fill=0.0,
                            base=0,
                            channel_multiplier=-1,
                            pattern=[[1, 128]])
                m_pe(2 * (valid * PE_CYC + MM_OH))
                m_exp((p, c, jt), valid)

            av_tiles = {}

            def setup_group(c, p):
                for l in range(2):
                    av_tiles[(c, p, l)] = ps_av.tile(
                        [128, CH], F32, tag="ps_av", name=f"av{c}p{p}l{l}")

            def av_batch(c, p, jts, heads=(0, 1)):
                njt = 4 * (c + 1)
                for l in heads:
                    h = p * 2 + l
                    for jt in jts:
                        jl = jt - 4 * c
                        off = 128 * jl if jl > 0 else 0
                        valid = CH - off
                        e_t = e_tiles[(p, c, jt)]
                        nc.tensor.matmul(
                            av_tiles[(c, p, l)][:, off:],
                            vh_sb[:, (jt * HPC + h), :, :],
                            e_t[:, l, off:],
                            start=(jt == 0),
                            stop=(jt == njt - 1),
                            skip_group_check=True)
                        m_pe(valid * PE_CYC + MM_OH)
                        e_consumed[(p, c, jt)] -= 1

            def fin_head(c, p, l, last=False):
                nonlocal avfree
                av_t = av_tiles[(c, p, l)]
                h = p * 2 + l
                sums_sb = sb_n.tile([64, CH], F32, tag="sums", name=f"sm{c}p{p}l{l}")
                if last:
                    nc.scalar.activation(sums_sb[:], av_t[64:128, :],
                                         mybir.ActivationFunctionType.Copy)
                else:
                    nc.vector.tensor_copy(sums_sb[:], av_t[64:128, :])
                rc = sb_n.tile([64, CH], F32, tag="rc", name=f"rc{c}p{p}l{l}")
                nc.vector.reciprocal_approx_fast(rc[:], sums_sb[:])
                out_sb = sb_n.tile([64, CH], F32, tag="out", name=f"ob{c}p{p}l{l}")
                nc.vector.tensor_mul(out_sb[:], av_t[0:64, :], rc[:])
                # final block: ACT's DGE queue is idle, issue there so the two
                # last out-DMAs go out in parallel with Sync's
                dq = nc.scalar if last else nc.sync
                dq.dma_start(
                    outT[h * 64:(h + 1) * 64, c * CH:(c + 1) * CH], out_sb[:])
                M['dve'] = max(M['dve'], M['pe']) + 1350.0
                avfree = M['dve']

            warm_n = [0]

            def dummy(n=1, cold=False):
                wp = ps_qkv.tile([128, CH], F32, tag="ps_qkv", name=f"wm{warm_n[0]}")
                warm_n[0] += 1
                for _ in range(n):
                    nc.tensor.matmul(wp[0:1, :], wzr[:], xzr[:],
                                     start=True, stop=True, skip_group_check=True)
                    m_pe(CH * (1 / 1.2 if cold else PE_CYC) + MM_OH)

            # ---------- prologue ----------
            # DMA order: wq, x0 (halved), wk -> q(0,*) can start right after the
            # fixed ~7us engine preamble; real q/k matmuls double as HAM warmup.
            dma_w("q")
            dma_x(0, halves=True)
            dma_w("k")
            dma_w("v")
            dma_x(1)
            nc.vector.memset(wz[:], 0.0)
            nc.vector.tensor_copy(wzr[:], wz[:])
            nc.vector.memset(xz[:], 0.0)
            nc.vector.tensor_copy(xzr[:], xz[:])
            M['pe'] = 7200.0   # engine preamble; DMA mostly overlaps it
            dummy(6, cold=True)  # lift HAM while the first input DMAs land

            # ---------- streams for the emission loop ----------
            # S stream: blocks (c,p), unit list (c,p,jt)
            S_stream = [(p, c, jt) for c in range(NCH) for p in range(2)
                        for jt in range(4 * (c + 1))]
            # AV stream: per block: setup, batches (groups of 4 jts; last block split), fins
            AV_stream = []  # items: ('setup',c,p) ('batch',c,p,jts) ('fin',c,p,l,last)
            for c in range(NCH):
                for p in range(2):
                    njt = 4 * (c + 1)
                    AV_stream.append(('setup', c, p))
                    batches = [list(range(j0, j0 + 4)) for j0 in range(0, njt - 4, 4)]
                    last = (c == 3 and p == 1)
                    if last:
                        # final block: head-separated tail so fin(l0) overlaps
                        # head l1's remaining AV matmuls
                        for bjts in batches:
                            AV_stream.append(('batch', c, p, bjts))
                        AV_stream.append(('batchh', c, p, [12, 13, 14], 0))
                        AV_stream.append(('batchh', c, p, [15], 0))
                        AV_stream.append(('fin', c, p, 0, True))
                        AV_stream.append(('batchh', c, p, [12, 13, 14], 1))
                        AV_stream.append(('batchh', c, p, [15], 1))
                        AV_stream.append(('fin', c, p, 1, True))
                    else:
                        for bjts in batches + [list(range(njt - 4, njt))]:
                            AV_stream.append(('batch', c, p, bjts))
                        AV_stream.append(('fin', c, p, 0, False))
                        AV_stream.append(('fin', c, p, 1, False))
            # filler queue (ordered); entries: ('qk',c,which,p) ('v',c,il)
            # ('vones',c) ('dmax',c)
            fillers = (
                [('qk', 0, "q", 1), ('qk', 0, "k", 1)]
                + [('vones', 0)] + [('v', 0, il) for il in range(4)]
                + [('qk', 1, "q", 0), ('qk', 1, "k", 0), ('qk', 1, "q", 1), ('qk', 1, "k", 1)]
                + [('dmax', 2)]
                + [('vones', 1)] + [('v', 1, il) for il in range(4)]
                + [('qk', 2, "q", 0), ('qk', 2, "k", 0), ('qk', 2, "q", 1), ('qk', 2, "k", 1)]
                + [('dmax', 3)]
                + [('vones', 2)] + [('v', 2, il) for il in range(4)]
                + [('qk', 3, "q", 0), ('qk', 3, "q", 1)]
                + [('qk', 3, "k", 0), ('qk', 3, "k", 1)]
                + [('vones', 3)] + [('v', 3, il) for il in range(4)]
            )
            qk_emitted = {(0, "q", 0): False, (0, "k", 0): False,
                          (0, "q", 1): False, (0, "k", 1): False}
            for f in fillers:
                if f[0] == 'qk':
                    qk_emitted[(f[1], f[2], f[3])] = False
            v_emitted = set()
            vones_emitted = set()
            dmax_emitted = {0, 1}

            def run_filler(f):
                if f[0] == 'qk':
                    _, c, which, p = f
                    qk_piece(c, which, p)
                    qk_emitted[(c, which, p)] = True
                elif f[0] == 'v':
                    _, c, il = f
                    v_piece(c, il)
                    v_emitted.add((c, il))
                elif f[0] == 'vones':
                    c = f[1]
                    nc.vector.memset(vh_sb[:, c * 16:(c + 1) * 16, 1, :], 1.0)
                    vones_emitted.add(c)
                else:
                    dma_x(f[1])
                    dmax_emitted.add(f[1])

            def filler_index_for_dep(dep):
                for i, f in enumerate(fillers):
                    if f == dep:
                        return i
                return None

            def force_dep(dep):
                """Emit fillers up to and including dep (preserving queue order)."""
                idx = filler_index_for_dep(dep)
                if idx is None:
                    return
                for f in fillers[:idx + 1]:
                    run_filler(f)
                del fillers[:idx + 1]

            # pair-0 chunk-0 q/k up front; pair-1 pieces fill S(0,0,*) stalls
            for (c, w, p) in [(0, "q", 0), (0, "k", 0)]:
                qk_piece(c, w, p)
                qk_emitted[(c, w, p)] = True

            # ---------- emission loop ----------
            si = 0
            ai = 0
            plan = []

            def s_ready_wait():
                """Modeled stall if the next S unit were emitted now (inf = dep missing)."""
                p, c, jt = S_stream[si]
                if not qk_emitted[(c, "q", p)] or not qk_emitted[(jt // 4, "k", p)]:
                    return None  # needs filler dep
                w = 0.0
                if len(sring) >= NBUF_S:
                    w = max(w, sring[-NBUF_S] - M['pe'])
                live = sum(1 for v in e_consumed.values() if v > 0)
                if live >= EBUFS - 1:
                    # wait until oldest live unit is consumed; approximate w/ large stall
                    w = max(w, 1e9)
                return w

            def av_ready_wait():
                item = AV_stream[ai]
                if item[0] == 'setup':
                    return 0.0
                if item[0] == 'fin':
                    return 0.0
                c, p, jts = item[1], item[2], item[3]
                for jt in jts:
                    if any((jt // 4, il) not in v_emitted for il in range(4)):
                        return None  # needs v filler
                    if (jt // 4) not in vones_emitted:
                        return None
                    if (p, c, jt) not in exp_done:
                        return 1e9
                w = max(0.0, avfree - M['pe'])
                if jts[0] == 0:
                    w = max(w, 0.0)
                w = max(w, exp_done[(p, c, jts[-1])] - M['pe'])
                return w

            while si < len(S_stream) or ai < len(AV_stream):
                sw = s_ready_wait() if si < len(S_stream) else 1e18
                aw = av_ready_wait() if ai < len(AV_stream) else 1e18
                # resolve missing deps by force-emitting fillers
                if sw is None and (aw is None or aw > 0):
                    p, c, jt = S_stream[si]
                    if not qk_emitted[(c, "q", p)]:
                        force_dep(('qk', c, "q", p))
                    if not qk_emitted[(jt // 4, "k", p)]:
                        force_dep(('qk', jt // 4, "k", p))
                    continue
                if aw is None and (sw is None or sw > 0):
                    item = AV_stream[ai]
                    if item[0] == 'batch':
                        cv = item[3][0] // 4
                        force_dep(('v', cv, 3))
                    continue
                sw = 1e18 if sw is None else sw
                aw = 1e18 if aw is None else aw
                act_backlog = M['act'] - M['pe']
                if sw <= 0 and act_backlog < 2.0 * 1147.0:
                    p, c, jt = S_stream[si]
                    s_exp(p, c, jt)
                    plan.append(('S', p, c, jt, M['pe']))
                    si += 1
                elif aw <= 0:
                    item = AV_stream[ai]
                    if item[0] == 'setup':
                        setup_group(item[1], item[2])
                    elif item[0] == 'fin':
                        fin_head(item[1], item[2], item[3], item[4])
                        plan.append(('fin', item[1], item[2], M['pe']))
                    elif item[0] == 'batchh':
                        av_batch(item[1], item[2], item[3], heads=(item[4],))
                        plan.append(('avb', item[1], item[2], tuple(item[3]), M['pe']))
                    else:
                        av_batch(item[1], item[2], item[3])
                        plan.append(('avb', item[1], item[2], tuple(item[3]), M['pe']))
                    ai += 1
                elif sw <= 0:
                    p, c, jt = S_stream[si]
                    s_exp(p, c, jt)
                    plan.append(('S', p, c, jt, M['pe']))
                    si += 1
                elif fillers:
                    run_filler(fillers.pop(0))
                    plan.append(('fill', M['pe']))
                else:
                    # dummy warm matmuls to cover the predicted stall
                    need = min(sw, aw)
                    nd = max(1, int(need / (CH * PE_CYC + MM_OH)) )
                    nd = min(nd, 8)
                    dummy(nd)
                    plan.append(('dummy', nd, M['pe']))

            while fillers:
                run_filler(fillers.pop(0))

            if debug_plan is not None:
                debug_plan.append((M['pe'], M['act'], plan))

    nc.compile()
    return nc


def _get_nc():
    global _CACHED_NC
    if _CACHED_NC is None:
        _CACHED_NC = build_nc()
    return _CACHED_NC


def make_in_maps(x, W_qkv):
    x = np.asarray(x, dtype=np.float32)
    W = np.asarray(W_qkv, dtype=np.float32)
    bf = ml_dtypes.bfloat16

    def block_w(w):       # [1024, 256] -> [128, 8, 256], w[t*128+p, :] -> [p, t, :]
        return np.ascontiguousarray(
            w.reshape(DT, 128, 256).transpose(1, 0, 2)).astype(bf)

    in_maps = []
    for core in range(8):
        b, hg = core // 4, core % 4
        cols = slice(hg * 256, (hg + 1) * 256)
        xtT = x[b].T  # [1024, 2048]
        # [p, c, t, ch]: xt[t*128+p, c*512+ch]
        xtb = np.ascontiguousarray(
            xtT.reshape(DT, 128, NCH, CH).transpose(1, 2, 0, 3)).astype(bf)
        in_maps.append({
            "xt": xtb,
            "wq": block_w(W[:, 0 * D:1 * D][:, cols]),
            "wk": block_w(W[:, 1 * D:2 * D][:, cols]),
            "wv": block_w(W[:, 2 * D:3 * D][:, cols]),
        })
    return in_maps


def kernel(x, W_qkv, _res_hook=None):
    nc = _get_nc()
    in_maps = make_in_maps(x, W_qkv)
    res = run_bass_kernel_spmd(nc, in_maps, list(range(8)))
    if _res_hook is not None:
        _res_hook(res)
    out = np.empty((B, N, D), dtype=np.float32)
    for core in range(8):
        b, hg = core // 4, core % 4
        out[b, :, hg * 256:(hg + 1) * 256] = res.results[core]["outT"].T
    return out


if __name__ == "__main__":
    dbg = []
    build_nc(debug_plan=dbg)
    pe, act, plan = dbg[0]
    print(f"model: PE end {pe/1000:.1f}us  ACT end {act/1000:.1f}us")
    nd = sum(p[1] for p in plan if p[0] == 'dummy')
    print(f"dummies: {nd}")


# revision 47
# speedup vs baseline: 1.0219x; 1.0219x over previous
"""Causal self-attention Trainium2 Bass kernel (bf16, causal-exact, model-driven schedule).

Problem: B=2, N=2048, H=16 heads, Dh=64, D=1024, fp32 in/out.
  qkv = x @ W_qkv; causal softmax(q k^T / sqrt(Dh)) @ v.

Sharding (8 cores): data-parallel on B (2) x tensor-parallel on head groups (4).
Core c handles batch b = c // 4 and heads hg*4 .. hg*4+3 where hg = c % 4.

Inputs are converted to bf16 AND pre-blocked on the host so every DMA is a
contiguous run per SBUF partition (128 descriptors of 4-8KB instead of 1024 of
1KB); weights ride the ACT hardware DGE queue while x chunks ride Sync's, so
the two DMA queues transfer in parallel during the prologue. PE matmul inputs
are bf16 (same 1 col/cycle streaming rate as f32r but fast-weight-load works
and SBUF/DMA halve; fp8 was measured numerically unacceptable: S-fp8 3.1e-2,
AV-fp8 1.9e-1 vs the 2e-2 gate). PSUM accumulation stays fp32.
Per-core layouts (no device transposes):
  xt  [128, 4, 8, 512] bf16; [p, c, t, :] = x[b].T[t*128+p, c*512:(c+1)*512]
  wq/wk/wv [128, 8, 256] bf16; [p, t, :] = W_slice[t*128+p, :]
  outT [256, 2048] f32; row h*64+d, col i = out[b, i, hg*256 + h*64 + d]

Device algorithm per core (all matmul inputs bf16, PSUM fp32):
  qT/kT [dh, i] via matmul(lhsT=W-slice, rhs=xT), PSUM -> bf16 SBUF
  v     [i, dh] via matmul(lhsT=xT-slice, rhs=Wv), stored v-hat = [v | ones64]
  S^T   [j, i] row-tiled matmul pairs (K=64/head, tile_position (l*64,0)) into a
        [128,2,512] PSUM tile; on diagonal j-tiles only the valid i-range
        (i >= j-tile start) is computed/exp'd/accumulated (causal-exact).
  expS^T via one ACT Exp per j-tile (strided [2,valid] AP), fused 1/sqrt(Dh),
        bf16 out; gpsimd affine_select zeroes the 128x128 diagonal triangle only.
  AV    out^T += vhat^T e^T in PSUM; rows 64:128 accumulate the softmax
        denominator (ones trick); normalization = copy + fast reciprocal + mul.
        No max-subtraction: S ~ N(0,1).

Scheduling: the PE executes strictly in issue order, so build_nc runs a greedy
emission loop over unit streams (S/exp units, AV batches, QKV/v/DMA fillers,
dummy warm matmuls) driven by an inline cost model (PE 2.4GHz warm, ACT 1.2GHz,
ps_s double-buffer backpressure, e-tile pool occupancy, PSUM bank group hand-off,
per-queue DMA completion). Dummy matmuls fill any predicted PE idle so the HAM
clock gate never re-throttles (PE micro-idle windows accumulate over a ~3.4us
monitor window and halve the PE clock; measured as the dominant cost of the
original schedule). The exp stream (ScalarE, 1 elem/cycle/lane, ~77us total)
is the pacing constraint late in the kernel: exp calls per j-tile stay at
[128, 2, valid] granularity - merging pairs of calls to save the 352-cycle
ACT overhead was tried and regressed 40% by killing the S/exp double-buffer.

Measured on HW: 197.1us (prev session baseline) -> 129.3us, rel err 4.9e-3.
PE busy ~112us of which ~99us is the streaming floor for this layout
(QKV 41us + S 29us + AV 29us at 1 result-col/cycle/128-rows, causal-exact).
"""

import numpy as np
import ml_dtypes

import concourse.mybir as mybir
import concourse.tile as tile
from concourse import bacc
from concourse.bass_utils import run_bass_kernel_spmd

F32 = mybir.dt.float32
BF16 = mybir.dt.bfloat16

B = 2
N = 2048
D = 1024
HPC = 4             # heads per core
DH = 64
NCH = 4             # i-chunks of 512
CH = 512
DT = 8              # d-tiles of 128
SCALE = 1.0 / 8.0   # 1/sqrt(64)

# ---- inline schedule model constants (ns) ----
PE_CYC = 1.0 / 2.4
ACT_CYC = 1.0 / 1.2
MM_OH = 14.0        # per-matmul issue overhead
ACT_OH = 352 * ACT_CYC
NBUF_S = 2          # ps_s pool depth (units)
EBUFS = 22          # sb_e pool depth (units)

_CACHED_NC = None


def build_nc(debug_plan=None):
    nc = bacc.Bacc("TRN2", target_bir_lowering=False, debug=False)
    # host pre-blocks inputs so every DMA is contiguous per partition:
    #   xt[p, c, t, :] = x[b].T[t*128+p, c*512:(c+1)*512]
    #   w*[p, t, :]    = W_slice[t*128+p, :]
    xt = nc.dram_tensor("xt", [128, NCH, DT, CH], BF16, kind="ExternalInput").ap()
    wq = nc.dram_tensor("wq", [128, DT, 256], BF16, kind="ExternalInput").ap()
    wk = nc.dram_tensor("wk", [128, DT, 256], BF16, kind="ExternalInput").ap()
    wv = nc.dram_tensor("wv", [128, DT, 256], BF16, kind="ExternalInput").ap()
    outT = nc.dram_tensor("outT", [HPC * DH, N], F32, kind="ExternalOutput").ap()

    with tile.TileContext(nc) as tc:
        with (
            tc.tile_pool(name="sb_w", bufs=1) as sb_w,
            tc.tile_pool(name="sb_x", bufs=3) as sb_x,
            tc.tile_pool(name="sb_qk", bufs=1) as sb_qk,
            tc.tile_pool(name="sb_v", bufs=1) as sb_v,
            tc.tile_pool(name="sb_e", bufs=EBUFS) as sb_e,
            tc.tile_pool(name="sb_n", bufs=6) as sb_n,
            tc.tile_pool(name="ps_av", bufs=2, space="PSUM") as ps_av,
            tc.tile_pool(name="ps_qkv", bufs=2, space="PSUM") as ps_qkv,
            tc.tile_pool(name="ps_s", bufs=NBUF_S, space="PSUM") as ps_s,
        ):
            # ---------- persistent SBUF ----------
            wq_sb = sb_w.tile([128, DT, 256], BF16)
            wk_sb = sb_w.tile([128, DT, 256], BF16)
            wv_sb = sb_w.tile([128, DT, 256], BF16)
            qt_sb = sb_qk.tile([128, 2 * N], BF16)   # pair-major, head l at rows l*64
            kt_sb = sb_qk.tile([128, 2 * N], BF16)
            # v-hat: [slot=(it*4+h)][v|ones][64]
            vh_sb = sb_v.tile([128, 16 * HPC, 2, 64], BF16)

            # zeros for dummy matmuls (memset f32 then cast - memset on bf16 is untested)
            wz = sb_v.tile([128, 1], F32)
            xz = sb_v.tile([128, CH], F32)
            wzr = sb_v.tile([128, 1], BF16)
            xzr = sb_v.tile([128, CH], BF16)

            # ---------- DMA thunks (contiguous-per-partition transfers) ----------
            # weights go out on the ACT hardware DGE queue, x chunks on Sync's:
            # the two queues transfer in parallel during the prologue.
            DMA_US_PER_MB = 5.6   # ~180 GB/s effective per queue
            dma_done = {}

            def _dma_model(queue, key, mbytes):
                t0 = max(M[queue], M['pe'], 7000.0)
                M[queue] = t0 + 600.0 + mbytes * DMA_US_PER_MB * 1000.0
                dma_done[key] = M[queue]

            def dma_w(which):
                w_sb, w_dram = {"q": (wq_sb, wq), "k": (wk_sb, wk), "v": (wv_sb, wv)}[which]
                nc.scalar.dma_start(w_sb[:, :, :], w_dram[:, :, :])
                _dma_model('actq', 'w' + which, 0.5)

            xtc_tiles = {}

            def dma_x(c, halves=False):
                xtc = sb_x.tile([128, DT, CH], BF16, tag="xtc", name=f"xtc{c}")
                xtc_tiles[c] = xtc
                if halves:
                    nc.sync.dma_start(xtc[:, 0:DT // 2, :], xt[:, c, 0:DT // 2, :])
                    _dma_model('synq', f'x{c}a', 0.5)
                    nc.sync.dma_start(xtc[:, DT // 2:DT, :], xt[:, c, DT // 2:DT, :])
                    _dma_model('synq', f'x{c}b', 0.5)
                else:
                    nc.sync.dma_start(xtc[:, :, :], xt[:, c, :, :])
                    _dma_model('synq', f'x{c}a', 1.0)
                    dma_done[f'x{c}b'] = dma_done[f'x{c}a']

            # ---------- schedule model state ----------
            M = dict(pe=0.0, act=0.0, dve=0.0, synq=0.0, actq=0.0, ncold=0)
            sring = []          # exp_done times of S units, for ps_s backpressure
            exp_done = {}       # (p,c,jt) -> modeled exp completion
            e_consumed = {}     # (p,c,jt) -> remaining AV consumers
            avfree = 0.0        # modeled time previous AV bank group is released

            def m_pe(dur):
                M['pe'] += dur

            def m_exp(key, valid):
                st = max(M['pe'], M['act'])
                M['act'] = st + 2 * valid * ACT_CYC + ACT_OH
                exp_done[key] = M['act']
                e_consumed[key] = 2
                sring.append(M['act'])

            # ---------- compute thunks ----------
            def fill_to(t):
                """Emit dummy matmuls to cover a predicted PE stall until time t."""
                stall = t - M['pe']
                if stall > 400.0:
                    dummy(min(24, max(1, int(stall / 380.0))))
                if t > M['pe']:
                    M['pe'] = t

            def qk_piece(c, which, p):
                xtc = xtc_tiles[c]
                w_sb, dst = (wq_sb, qt_sb) if which == "q" else (wk_sb, kt_sb)
                wkey = 'wq' if which == "q" else 'wk'
                a1 = max(dma_done.get(wkey, 0.0), dma_done.get(f'x{c}a', 0.0))
                a2 = dma_done.get(f'x{c}b', 0.0)
                fill_to(a1)
                pres = ps_qkv.tile([128, CH], F32, tag="ps_qkv", name=f"{which}{c}p{p}")
                for t in range(DT // 2):
                    nc.tensor.matmul(
                        pres[:], w_sb[:, t, p * 128:(p + 1) * 128],
                        xtc[:, t, :],
                        start=(t == 0), stop=False)
                m_pe((DT // 2) * (CH * PE_CYC + MM_OH))
                fill_to(a2)
                for t in range(DT // 2, DT):
                    nc.tensor.matmul(
                        pres[:], w_sb[:, t, p * 128:(p + 1) * 128],
                        xtc[:, t, :],
                        start=False, stop=(t == DT - 1))
                m_pe((DT // 2) * (CH * PE_CYC + MM_OH))
                nc.vector.tensor_copy(dst[:, p * N + c * CH: p * N + (c + 1) * CH], pres[:])

            def v_piece(c, il):
                xtc = xtc_tiles[c]
                it = 4 * c + il
                fill_to(max(dma_done.get('wv', 0.0), dma_done.get(f'x{c}b', 0.0)))
                v_ps = ps_qkv.tile([128, 256], F32, tag="ps_qkv", name=f"v{c}i{il}")
                for t in range(DT):
                    nc.tensor.matmul(
                        v_ps[:], xtc[:, t, il * 128:(il + 1) * 128],
                        wv_sb[:, t, :],
                        start=(t == 0), stop=(t == DT - 1))
                for h in range(HPC):
                    nc.vector.tensor_copy(vh_sb[:, it * HPC + h, 0, :],
                                          v_ps[:, h * 64:(h + 1) * 64])
                m_pe(DT * (256 * PE_CYC + MM_OH))

            e_tiles = {}

            def s_exp(p, c, jt):
                jl = jt - 4 * c
                off = 128 * jl if jl > 0 else 0
                valid = CH - off
                s_ps = ps_s.tile([128, 2, CH], F32, tag="ps_s", name=f"s{c}p{p}j{jt}")
                for l in range(2):
                    nc.tensor.matmul(
                        s_ps[:, l, off:],
                        kt_sb[l * 64:(l + 1) * 64, p * N + jt * 128: p * N + (jt + 1) * 128],
                        qt_sb[l * 64:(l + 1) * 64, p * N + c * CH + off: p * N + (c + 1) * CH],
                        start=True, stop=True,
                        tile_position=(l * 64, 0))
                e_t = sb_e.tile([128, 2, CH], BF16, tag="e", name=f"e{c}p{p}j{jt}")
                e_tiles[(p, c, jt)] = e_t
                nc.scalar.activation(e_t[:, :, off:], s_ps[:, :, off:],
                                     mybir.ActivationFunctionType.Exp,
                                     scale=SCALE)
                if jl >= 0:  # diagonal tile: zero where j > i inside the 128-wide square
                    for l in range(2):
                        nc.gpsimd.affine_select(
                            out=e_t[:, l, off:off + 128],
                            in_=e_t[:, l, off:off + 128],
                            compare_op=mybir.AluOpType.is_ge,
                            fill=0.0,
                            base=0,
                            channel_multiplier=-1,
                            pattern=[[1, 128]])
                m_pe(2 * (valid * PE_CYC + MM_OH))
                m_exp((p, c, jt), valid)

            av_tiles = {}

            def setup_group(c, p):
                for l in range(2):
                    av_tiles[(c, p, l)] = ps_av.tile(
                        [128, CH], F32, tag="ps_av", name=f"av{c}p{p}l{l}")

            def av_batch(c, p, jts, heads=(0, 1)):
                njt = 4 * (c + 1)
                for l in heads:
                    h = p * 2 + l
                    for jt in jts:
                        jl = jt - 4 * c
                        off = 128 * jl if jl > 0 else 0
                        valid = CH - off
                        e_t = e_tiles[(p, c, jt)]
                        nc.tensor.matmul(
                            av_tiles[(c, p, l)][:, off:],
                            vh_sb[:, (jt * HPC + h), :, :],
                            e_t[:, l, off:],
                            start=(jt == 0),
                            stop=(jt == njt - 1),
                            skip_group_check=True)
                        m_pe(valid * PE_CYC + MM_OH)
                        e_consumed[(p, c, jt)] -= 1

            def fin_head(c, p, l, last=False):
                nonlocal avfree
                av_t = av_tiles[(c, p, l)]
                h = p * 2 + l
                sums_sb = sb_n.tile([64, CH], F32, tag="sums", name=f"sm{c}p{p}l{l}")
                if last:
                    nc.scalar.activation(sums_sb[:], av_t[64:128, :],
                                         mybir.ActivationFunctionType.Copy)
                else:
                    nc.vector.tensor_copy(sums_sb[:], av_t[64:128, :])
                rc = sb_n.tile([64, CH], F32, tag="rc", name=f"rc{c}p{p}l{l}")
                nc.vector.reciprocal_approx_fast(rc[:], sums_sb[:])
                out_sb = sb_n.tile([64, CH], F32, tag="out", name=f"ob{c}p{p}l{l}")
                nc.vector.tensor_mul(out_sb[:], av_t[0:64, :], rc[:])
                nc.sync.dma_start(
                    outT[h * 64:(h + 1) * 64, c * CH:(c + 1) * CH], out_sb[:])
                M['dve'] = max(M['dve'], M['pe']) + 1350.0
                avfree = M['dve']

            warm_n = [0]

            def dummy(n=1, cold=False):
                wp = ps_qkv.tile([128, CH], F32, tag="ps_qkv", name=f"wm{warm_n[0]}")
                warm_n[0] += 1
                for _ in range(n):
                    nc.tensor.matmul(wp[0:1, :], wzr[:], xzr[:],
                                     start=True, stop=True, skip_group_check=True)
                    m_pe(CH * (1 / 1.2 if cold else PE_CYC) + MM_OH)

            # ---------- prologue ----------
            # DMA order: wq, x0 (halved), wk -> q(0,*) can start right after the
            # fixed ~7us engine preamble; real q/k matmuls double as HAM warmup.
            dma_w("q")
            dma_x(0, halves=True)
            dma_w("k")
            dma_w("v")
            dma_x(1)
            nc.vector.memset(wz[:], 0.0)
            nc.vector.tensor_copy(wzr[:], wz[:])
            nc.vector.memset(xz[:], 0.0)
            nc.vector.tensor_copy(xzr[:], xz[:])
            M['pe'] = 7200.0   # engine preamble; DMA mostly overlaps it
            dummy(6, cold=True)  # lift HAM while the first input DMAs land

            # ---------- streams for the emission loop ----------
            # S stream: blocks (c,p), unit list (c,p,jt)
            S_stream = [(p, c, jt) for c in range(NCH) for p in range(2)
                        for jt in range(4 * (c + 1))]
            # AV stream: per block: setup, batches (groups of 4 jts; last block split), fins
            AV_stream = []  # items: ('setup',c,p) ('batch',c,p,jts) ('fin',c,p,l,last)
            for c in range(NCH):
                for p in range(2):
                    njt = 4 * (c + 1)
                    AV_stream.append(('setup', c, p))
                    batches = [list(range(j0, j0 + 4)) for j0 in range(0, njt - 4, 4)]
                    last = (c == 3 and p == 1)
                    if last:
                        # final block: head-separated tail so fin(l0) overlaps
                        # head l1's remaining AV matmuls
                        for bjts in batches:
                            AV_stream.append(('batch', c, p, bjts))
                        AV_stream.append(('batchh', c, p, [12, 13, 14], 0))
                        AV_stream.append(('batchh', c, p, [15], 0))
                        AV_stream.append(('fin', c, p, 0, True))
                        AV_stream.append(('batchh', c, p, [12, 13, 14], 1))
                        AV_stream.append(('batchh', c, p, [15], 1))
                        AV_stream.append(('fin', c, p, 1, True))
                    else:
                        for bjts in batches + [list(range(njt - 4, njt))]:
                            AV_stream.append(('batch', c, p, bjts))
                        AV_stream.append(('fin', c, p, 0, False))
                        AV_stream.append(('fin', c, p, 1, False))
            # filler queue (ordered); entries: ('qk',c,which,p) ('v',c,il)
            # ('vones',c) ('dmax',c)
            fillers = (
                [('qk', 0, "q", 1), ('qk', 0, "k", 1)]
                + [('vones', 0)] + [('v', 0, il) for il in range(4)]
                + [('qk', 1, "q", 0), ('qk', 1, "k", 0), ('qk', 1, "q", 1), ('qk', 1, "k", 1)]
                + [('dmax', 2)]
                + [('vones', 1)] + [('v', 1, il) for il in range(4)]
                + [('qk', 2, "q", 0), ('qk', 2, "k", 0), ('qk', 2, "q", 1), ('qk', 2, "k", 1)]
                + [('dmax', 3)]
                + [('vones', 2)] + [('v', 2, il) for il in range(4)]
                + [('qk', 3, "q", 0), ('qk', 3, "q", 1)]
                + [('qk', 3, "k", 0), ('qk', 3, "k", 1)]
                + [('vones', 3)] + [('v', 3, il) for il in range(4)]
            )
            qk_emitted = {(0, "q", 0): False, (0, "k", 0): False,
                          (0, "q", 1): False, (0, "k", 1): False}
            for f in fillers:
                if f[0] == 'qk':
                    qk_emitted[(f[1], f[2], f[3])] = False
            v_emitted = set()
            vones_emitted = set()
            dmax_emitted = {0, 1}

            def run_filler(f):
                if f[0] == 'qk':
                    _, c, which, p = f
                    qk_piece(c, which, p)
                    qk_emitted[(c, which, p)] = True
                elif f[0] == 'v':
                    _, c, il = f
                    v_piece(c, il)
                    v_emitted.add((c, il))
                elif f[0] == 'vones':
                    c = f[1]
                    nc.vector.memset(vh_sb[:, c * 16:(c + 1) * 16, 1, :], 1.0)
                    vones_emitted.add(c)
                else:
                    dma_x(f[1])
                    dmax_emitted.add(f[1])

            def filler_index_for_dep(dep):
                for i, f in enumerate(fillers):
                    if f == dep:
                        return i
                return None

            def force_dep(dep):
                """Emit fillers up to and including dep (preserving queue order)."""
                idx = filler_index_for_dep(dep)
                if idx is None:
                    return
                for f in fillers[:idx + 1]:
                    run_filler(f)
                del fillers[:idx + 1]

            # pair-0 chunk-0 q/k up front; pair-1 pieces fill S(0,0,*) stalls
            for (c, w, p) in [(0, "q", 0), (0, "k", 0)]:
                qk_piece(c, w, p)
                qk_emitted[(c, w, p)] = True

            # ---------- emission loop ----------
            si = 0
            ai = 0
            plan = []

            def s_ready_wait():
                """Modeled stall if the next S unit were emitted now (inf = dep missing)."""
                p, c, jt = S_stream[si]
                if not qk_emitted[(c, "q", p)] or not qk_emitted[(jt // 4, "k", p)]:
                    return None  # needs filler dep
                w = 0.0
                if len(sring) >= NBUF_S:
                    w = max(w, sring[-NBUF_S] - M['pe'])
                live = sum(1 for v in e_consumed.values() if v > 0)
                if live >= EBUFS - 1:
                    # wait until oldest live unit is consumed; approximate w/ large stall
                    w = max(w, 1e9)
                return w

            def av_ready_wait():
                item = AV_stream[ai]
                if item[0] == 'setup':
                    return 0.0
                if item[0] == 'fin':
                    return 0.0
                c, p, jts = item[1], item[2], item[3]
                for jt in jts:
                    if any((jt // 4, il) not in v_emitted for il in range(4)):
                        return None  # needs v filler
                    if (jt // 4) not in vones_emitted:
                        return None
                    if (p, c, jt) not in exp_done:
                        return 1e9
                w = max(0.0, avfree - M['pe'])
                if jts[0] == 0:
                    w = max(w, 0.0)
                w = max(w, exp_done[(p, c, jts[-1])] - M['pe'])
                return w

            while si < len(S_stream) or ai < len(AV_stream):
                sw = s_ready_wait() if si < len(S_stream) else 1e18
                aw = av_ready_wait() if ai < len(AV_stream) else 1e18
                # resolve missing deps by force-emitting fillers
                if sw is None and (aw is None or aw > 0):
                    p, c, jt = S_stream[si]
                    if not qk_emitted[(c, "q", p)]:
                        force_dep(('qk', c, "q", p))
                    if not qk_emitted[(jt // 4, "k", p)]:
                        force_dep(('qk', jt // 4, "k", p))
                    continue
                if aw is None and (sw is None or sw > 0):
                    item = AV_stream[ai]
                    if item[0] == 'batch':
                        cv = item[3][0] // 4
                        force_dep(('v', cv, 3))
                    continue
                sw = 1e18 if sw is None else sw
                aw = 1e18 if aw is None else aw
                act_backlog = M['act'] - M['pe']
                if sw <= 0 and act_backlog < 1.6 * 1147.0:
                    p, c, jt = S_stream[si]
                    s_exp(p, c, jt)
                    plan.append(('S', p, c, jt, M['pe']))
                    si += 1
                elif aw <= 0:
                    item = AV_stream[ai]
                    if item[0] == 'setup':
                        setup_group(item[1], item[2])
                    elif item[0] == 'fin':
                        fin_head(item[1], item[2], item[3], item[4])
                        plan.append(('fin', item[1], item[2], M['pe']))
                    elif item[0] == 'batchh':
                        av_batch(item[1], item[2], item[3], heads=(item[4],))
                        plan.append(('avb', item[1], item[2], tuple(item[3]), M['pe']))
                    else:
                        av_batch(item[1], item[2], item[3])
                        plan.append(('avb', item[1], item[2], tuple(item[3]), M['pe']))
                    ai += 1
                elif sw <= 0:
                    p, c, jt = S_stream[si]
                    s_exp(p, c, jt)
                    plan.append(('S', p, c, jt, M['pe']))
                    si += 1
                elif fillers:
                    run_filler(fillers.pop(0))
                    plan.append(('fill', M['pe']))
                else:
                    # dummy warm matmuls to cover the predicted stall
                    need = min(sw, aw)
                    nd = max(1, int(need / (CH * PE_CYC + MM_OH)) )
                    nd = min(nd, 8)
                    dummy(nd)
                    plan.append(('dummy', nd, M['pe']))

            while fillers:
                run_filler(fillers.pop(0))

            if debug_plan is not None:
                debug_plan.append((M['pe'], M['act'], plan))

    nc.compile()
    return nc


def _get_nc():
    global _CACHED_NC
    if _CACHED_NC is None:
        _CACHED_NC = build_nc()
    return _CACHED_NC


def make_in_maps(x, W_qkv):
    x = np.asarray(x, dtype=np.float32)
    W = np.asarray(W_qkv, dtype=np.float32)
    bf = ml_dtypes.bfloat16

    def block_w(w):       # [1024, 256] -> [128, 8, 256], w[t*128+p, :] -> [p, t, :]
        return np.ascontiguousarray(
            w.reshape(DT, 128, 256).transpose(1, 0, 2)).astype(bf)

    in_maps = []
    for core in range(8):
        b, hg = core // 4, core % 4
        cols = slice(hg * 256, (hg + 1) * 256)
        xtT = x[b].T  # [1024, 2048]
        # [p, c, t, ch]: xt[t*128+p, c*512+ch]
        xtb = np.ascontiguousarray(
            xtT.reshape(DT, 128, NCH, CH).transpose(1, 2, 0, 3)).astype(bf)
        in_maps.append({
            "xt": xtb,
            "wq": block_w(W[:, 0 * D:1 * D][:, cols]),
            "wk": block_w(W[:, 1 * D:2 * D][:, cols]),
            "wv": block_w(W[:, 2 * D:3 * D][:, cols]),
        })
    return in_maps


def kernel(x, W_qkv, _res_hook=None):
    nc = _get_nc()
    in_maps = make_in_maps(x, W_qkv)
    res = run_bass_kernel_spmd(nc, in_maps, list(range(8)))
    if _res_hook is not None:
        _res_hook(res)
    out = np.empty((B, N, D), dtype=np.float32)
    for core in range(8):
        b, hg = core // 4, core % 4
        out[b, :, hg * 256:(hg + 1) * 256] = res.results[core]["outT"].T
    return out


if __name__ == "__main__":
    dbg = []
    build_nc(debug_plan=dbg)
    pe, act, plan = dbg[0]
    print(f"model: PE end {pe/1000:.1f}us  ACT end {act/1000:.1f}us")
    nd = sum(p[1] for p in plan if p[0] == 'dummy')
    print(f"dummies: {nd}")
